# revision 1
# baseline (speedup 1.0000x reference)
"""Gumbel Top-K gate kernel for Trainium2 (8 NeuronCores, SPMD).

Math: mask[b, 0, r, m] = 1 iff z[b, r, m] is among the top-16 of row r, where
  z = mean_h(q_h k_h^T)/sqrt(64) + gumbel(u),  gumbel = -log(-log(u+eps)+eps).
Softmax is strictly monotone per row, so the reference's softmax/top-k mask
equals thresholding z at its 16th-largest value per row (ties included via >=).

Sharding: core c handles batch b = c//2, row half c%2 -> [1024, 2048] slab.
Head-mean folds into one [1024, 512] x [512, 2048] matmul per core (concat
heads along the contraction dim). Host prep hands each core d-major (already
transposed) qT [512, 1024] (pre-scaled by the exact power-of-two 1/64 =
1/sqrt(64) * 1/8 head-mean) and kT [512, 2048], so the PE does zero
transposes.

Engine split per 128-row tile: PE: 16 accumulating fp32 score matmuls;
ACT: two Ln passes for the gumbel; DVE: z = S - g2 (PSUM read), the top-16
threshold (max8 -> match_replace -> max8), and the >= compare writing a
uint8 mask (exact 0/1; widened to f32 on host).
"""

import sys

sys.path.insert(0, "/opt/trn_rl_repo")

import numpy as np

import concourse.bacc as bacc
import concourse.mybir as mybir
import concourse.tile as tile
from concourse import bass_utils

B, H, N, D = 4, 8, 2048, 64
HD = H * D  # 512 contraction dim (heads concatenated)
N_CORES = 8
ROWS = N * B // N_CORES  # 1024 rows per core
P = 128
EPS = 1e-9
NEG_BIG = -3.0e38
F32 = mybir.dt.float32
F32R = mybir.dt.float32r
U8 = mybir.dt.uint8


def _make_identity(nc, ident, fill):
    nc.gpsimd.memset(ident, 0.0)
    sq = ident.shape[0]
    nc.gpsimd.affine_select(
        out=ident,
        in_=ident,
        compare_op=mybir.AluOpType.not_equal,
        fill=fill,
        base=0,
        pattern=[[-1, sq]],
        channel_multiplier=1,
    )


def _build_body(tc, qT_d, kT_d, u_d, mask_d):
    nc = tc.nc
    n_rtiles = ROWS // P  # 8
    n_c = HD // P  # 4 contraction chunks
    act = mybir.ActivationFunctionType

    with (
        tc.tile_pool(name="consts", bufs=1) as consts,
        tc.tile_pool(name="kqT", bufs=1) as kqT_pool,
        tc.tile_pool(name="s_psum", bufs=2, space="PSUM") as s_psum,
        tc.tile_pool(name="work", bufs=2) as work,
        tc.tile_pool(name="uin", bufs=3) as uin,
        tc.tile_pool(name="mout", bufs=2) as mout,
        tc.tile_pool(name="small", bufs=2) as small,
    ):
        eps_tile = consts.tile([P, 1], F32)
        nc.vector.memset(eps_tile, EPS)

        u_t = u_d.rearrange("(t p) n -> t p n", p=P)
        mask_t = mask_d.rearrange("(t p) n -> t p n", p=P)
        # prefetch tile 0's noise ahead of the weight loads so ACT starts early
        ut0 = uin.tile([P, N], F32, tag="u")
        nc.sync.dma_start(out=ut0, in_=u_t[0])

        # d-major loads straight from host-transposed DRAM; no PE transposes.
        # One DMA per 128-d chunk so c=0 matmuls start after 1 MiB, not 6 MiB.
        kT_r = kT_d.rearrange("(c p) m -> c p m", p=P)
        qT_r = qT_d.rearrange("(c p) m -> c p m", p=P)
        kT = [kqT_pool.tile([P, N], F32, tag=f"kT{c}", name=f"kT{c}") for c in range(n_c)]
        qT = [kqT_pool.tile([P, ROWS], F32, tag=f"qT{c}", name=f"qT{c}") for c in range(n_c)]
        for c in range(n_c):
            nc.sync.dma_start(out=kT[c], in_=kT_r[c])
            nc.sync.dma_start(out=qT[c], in_=qT_r[c])

        for t in range(n_rtiles):
            if t == 0:
                ut = ut0
            else:
                ut = uin.tile([P, N], F32, tag="u")
                nc.sync.dma_start(out=ut, in_=u_t[t])
            g1 = work.tile([P, N], F32, tag="g1")
            nc.scalar.activation(g1, ut, act.Ln, bias=eps_tile, scale=1.0)
            # g2 = log(-log(u+eps)+eps); z = S - g2
            g2 = work.tile([P, N], F32, tag="g2")
            nc.scalar.activation(g2, g1, act.Ln, bias=eps_tile, scale=-1.0)

            S = s_psum.tile([P, N], F32, tag="S")  # 4 PSUM banks
            for c in range(n_c):
                for m in range(4):
                    nc.tensor.matmul(
                        S[:, m * 512 : (m + 1) * 512],
                        qT[c][:, t * P : (t + 1) * P],
                        kT[c][:, m * 512 : (m + 1) * 512],
                        start=(c == 0),
                        stop=(c == n_c - 1),
                    )

            z = work.tile([P, N], F32, tag="z")
            nc.vector.tensor_sub(z, S, g2)  # PSUM read + gumbel add on DVE

            m8a = small.tile([P, 8], F32, tag="m8a")
            nc.vector.max(out=m8a, in_=z)
            zs = work.tile([P, N], F32, tag="zs")
            nc.vector.match_replace(
                out=zs, in_to_replace=m8a, in_values=z, imm_value=NEG_BIG
            )
            m8b = small.tile([P, 8], F32, tag="m8b")
            nc.vector.max(out=m8b, in_=zs)

            mk = mout.tile([P, N], U8, tag="mk")
            nc.vector.tensor_scalar(
                out=mk,
                in0=z,
                scalar1=m8b[:, 7:8],
                scalar2=None,
                op0=mybir.AluOpType.is_ge,
            )
            nc.sync.dma_start(out=mask_t[t], in_=mk)


def build_kernel():
    nc = bacc.Bacc(
        "TRN2", target_bir_lowering=False, debug=False, num_devices=N_CORES
    )
    qT = nc.dram_tensor("qT", [HD, ROWS], F32, kind="ExternalInput").ap()
    kT = nc.dram_tensor("kT", [HD, N], F32, kind="ExternalInput").ap()
    u = nc.dram_tensor("u", [ROWS, N], F32, kind="ExternalInput").ap()
    mask = nc.dram_tensor("mask", [ROWS, N], U8, kind="ExternalOutput").ap()
    with tile.TileContext(nc) as tc:
        _build_body(tc, qT, kT, u, mask)
    nc.compile()
    return nc


_NC_CACHE = None
LAST_RESULTS = None


def _get_nc():
    global _NC_CACHE
    if _NC_CACHE is None:
        _NC_CACHE = build_kernel()
    return _NC_CACHE


def make_in_maps(q, k, u):
    q = np.asarray(q, np.float32)
    k = np.asarray(k, np.float32)
    u = np.asarray(u, np.float32)
    in_maps = []
    kT_by_batch = {}
    for core in range(N_CORES):
        b, half = divmod(core, 2)
        r0 = half * ROWS
        if b not in kT_by_batch:
            # [N, H, D] -> [H*D, N] d-major
            kT_by_batch[b] = np.ascontiguousarray(
                k[b].transpose(1, 0, 2).reshape(N, HD).T
            )
        # 1/64 scale is an exact power-of-two: bit-identical to on-chip scaling
        qT = np.ascontiguousarray(
            q[b, :, r0 : r0 + ROWS, :].transpose(1, 0, 2).reshape(ROWS, HD).T
            * np.float32(1.0 / 64)
        )
        in_maps.append(
            {
                "qT": qT,
                "kT": kT_by_batch[b],
                "u": np.ascontiguousarray(u[b, r0 : r0 + ROWS]),
            }
        )
    return in_maps


def kernel(q, k, u):
    global LAST_RESULTS
    in_maps = make_in_maps(q, k, u)
    res = bass_utils.run_bass_kernel_spmd(
        _get_nc(), in_maps, core_ids=list(range(N_CORES))
    )
    LAST_RESULTS = res
    out = np.empty((B, 1, N, N), np.float32)
    for core in range(N_CORES):
        b, half = divmod(core, 2)
        r0 = half * ROWS
        out[b, 0, r0 : r0 + ROWS] = res.results[core]["mask"].astype(np.float32)
    return out

